# revision 7
# baseline (speedup 1.0000x reference)
"""Bidirectional GRU encoder (Keras reset_after=True, gates z,r,h) on 8 trn2 cores.

Sharding: 2 directions x 4-way batch split (B=64 -> 16 per core), zero collectives.
Each core embeds nothing (host gathers emb rows), computes its input projection
xg^T = W^T @ x^T on device, then runs the full T=128 recurrence for its batch
slice in a transposed layout: hidden state h^T kept as [U(part), B] so the
recurrent matmul is  hg^T[3U,B] = U^T @ h^T  with U tiles stationary (bf16 ->
FWL fast weight load) and h^T streaming.  No per-step transposes, no collectives.

Shapes (hardcoded): B=64 T=128 V=20000 E=512 U=1024, 3U=3072.
Per core: BC=16, tokens TB=T*BC=2048.
"""

import os

import numpy as np
import ml_dtypes

import concourse.bass as bass
from concourse import bacc
import concourse.mybir as mybir
import concourse.tile as tile
from concourse.bass_utils import run_bass_kernel_spmd

B, T, V, E, U = 64, 128, 20000, 512, 1024
G3 = 3 * U           # 3072
BC = 16              # batch per core
TB = T * BC          # 2048 tokens per core
KE = E // 128        # 4  k-tiles for input proj
KU = U // 128        # 8  k-tiles for recurrence
MG = G3 // 128       # 24 m-tiles of 3U
NCH = TB // 512      # 4  512-token chunks for the GEMM

F32 = mybir.dt.float32
BF16 = mybir.dt.bfloat16

_CACHE = {}


def _build_program():
    nc = bacc.Bacc(None, target_bir_lowering=False)

    xT_d = nc.dram_tensor("xT", [128, KE, TB], BF16, kind="ExternalInput")
    w_d = nc.dram_tensor("w", [128, KE, G3], BF16, kind="ExternalInput")
    u_d = nc.dram_tensor("u", [128, KU, G3], BF16, kind="ExternalInput")
    bias_d = nc.dram_tensor("bias", [1, G3], F32, kind="ExternalInput")
    bhh_d = nc.dram_tensor("bhh", [128, KU], F32, kind="ExternalInput")
    outs_d = nc.dram_tensor("outs", [T, 128, KU, BC], F32, kind="ExternalOutput")

    with tile.TileContext(nc) as tc:
        with (
            tc.tile_pool(name="persist", bufs=1) as pp,
            tc.tile_pool(name="small", bufs=1) as sp,
        ):
            xg_sb = pp.tile([128, MG, TB], BF16)      # 96KB/part
            u_sb = pp.tile([128, KU, G3], BF16)       # 48KB/part
            bias_sb = sp.tile([1, G3], F32)
            bhh_sb = sp.tile([128, KU], F32)
            ones_sb = sp.tile([1, 512], F32)

            nc.sync.dma_start(u_sb[:], u_d[:])
            nc.sync.dma_start(bias_sb[:], bias_d[:])
            nc.sync.dma_start(bhh_sb[:], bhh_d[:])
            nc.vector.memset(ones_sb[:], 1.0)

            # ---------------- Phase 1: xg^T = W^T @ x^T (+bias) ----------------
            with (
                tc.tile_pool(name="gemm", bufs=1) as gp,
                tc.tile_pool(name="wstream", bufs=3) as wp,
                tc.tile_pool(name="gpsum", bufs=2, space="PSUM") as gps,
            ):
                xt_sb = gp.tile([128, KE, TB], BF16)  # 16KB/part
                nc.sync.dma_start(xt_sb[:], xT_d[:])
                for m in range(MG):
                    w_sb = wp.tile([128, KE, 128], BF16, tag="wtile")
                    nc.sync.dma_start(w_sb[:], w_d[:, :, m * 128:(m + 1) * 128])
                    for nch in range(NCH):
                        ps = gps.tile([128, 512], F32, tag="gps")
                        for k in range(KE):
                            nc.tensor.matmul(
                                ps[:],
                                w_sb[:, k, :],
                                xt_sb[:, k, nch * 512:(nch + 1) * 512],
                                start=(k == 0),
                                stop=False,
                            )
                        nc.tensor.matmul(
                            ps[:],
                            bias_sb[:, m * 128:(m + 1) * 128],
                            ones_sb[:],
                            start=False,
                            stop=True,
                        )
                        nc.vector.tensor_copy(
                            out=xg_sb[:, m, nch * 512:(nch + 1) * 512],
                            in_=ps[:],
                        )

            # ---------------- Phase 2: recurrence ----------------
            with (
                tc.tile_pool(name="hpool", bufs=2) as hp,
                tc.tile_pool(name="gate", bufs=2) as gt,
                tc.tile_pool(name="spsum", bufs=2, space="PSUM") as sps,
            ):
                h32 = hp.tile([128, KU, BC], F32, tag="h32")
                hbf = hp.tile([128, KU, BC], BF16, tag="hbf")
                nc.vector.memset(h32[:], 0.0)
                nc.vector.memset(hbf[:], 0.0)

                for t in range(T):
                    psz = sps.tile([128, KU, BC], F32, tag="psz")
                    psr = sps.tile([128, KU, BC], F32, tag="psr")
                    psh = sps.tile([128, KU, BC], F32, tag="psh")
                    for m in range(MG):
                        ps, j = (
                            (psz, m) if m < 8 else
                            (psr, m - 8) if m < 16 else
                            (psh, m - 16)
                        )
                        for k in range(KU):
                            nc.tensor.matmul(
                                ps[:, j, :],
                                u_sb[:, k, m * 128:(m + 1) * 128],
                                hbf[:, k, :],
                                start=(k == 0),
                                stop=(k == KU - 1),
                            )
                    ts_ = slice(t * BC, (t + 1) * BC)
                    xz = xg_sb[:, 0:8, ts_]
                    xr = xg_sb[:, 8:16, ts_]
                    xh = xg_sb[:, 16:24, ts_]

                    z = gt.tile([128, KU, BC], F32, tag="z")
                    r = gt.tile([128, KU, BC], F32, tag="r")
                    nc.vector.tensor_add(out=z[:], in0=psz[:], in1=xz)
                    nc.scalar.activation(z[:], z[:], mybir.ActivationFunctionType.Sigmoid)
                    nc.vector.tensor_add(out=r[:], in0=psr[:], in1=xr)
                    nc.scalar.activation(r[:], r[:], mybir.ActivationFunctionType.Sigmoid)

                    t0 = gt.tile([128, KU, BC], F32, tag="t0")
                    nc.vector.tensor_add(
                        out=t0[:], in0=psh[:],
                        in1=bhh_sb[:, :, None].to_broadcast((128, KU, BC)),
                    )
                    nc.vector.tensor_mul(out=t0[:], in0=r[:], in1=t0[:])
                    nc.vector.tensor_add(out=t0[:], in0=t0[:], in1=xh)
                    hc = gt.tile([128, KU, BC], F32, tag="hc")
                    nc.scalar.activation(hc[:], t0[:], mybir.ActivationFunctionType.Tanh)

                    d = gt.tile([128, KU, BC], F32, tag="d")
                    nc.vector.tensor_sub(out=d[:], in0=h32[:], in1=hc[:])
                    h32 = hp.tile([128, KU, BC], F32, tag="h32")
                    nc.vector.tensor_mul(out=h32[:], in0=z[:], in1=d[:])
                    nc.vector.tensor_add(out=h32[:], in0=hc[:], in1=h32[:])
                    hbf = hp.tile([128, KU, BC], BF16, tag="hbf")
                    nc.vector.tensor_copy(out=hbf[:], in_=h32[:])
                    nc.sync.dma_start(outs_d[t], h32[:])

    nc.compile()
    return nc


def _prep_core_inputs(x_dir, w2, u2, bi, bh):
    """x_dir: [BC, T, E] f32 (already direction-ordered). Returns input map."""
    xT = np.ascontiguousarray(x_dir.transpose(2, 1, 0))        # [E, T, BC]
    xT = xT.reshape(KE, 128, TB).transpose(1, 0, 2)            # [128, KE, TB]
    w = w2.reshape(KE, 128, G3).transpose(1, 0, 2)             # [128, KE, G3]
    u = u2.reshape(KU, 128, G3).transpose(1, 0, 2)             # [128, KU, G3]
    bias = bi.astype(np.float64) + np.concatenate([bh[:2 * U], np.zeros(U)])
    bias = bias.astype(np.float32).reshape(1, G3)              # [1, G3]
    bhh = bh[2 * U:].astype(np.float32).reshape(KU, 128).T     # [128, KU]
    bf = ml_dtypes.bfloat16
    return {
        "xT": np.ascontiguousarray(xT).astype(bf),
        "w": np.ascontiguousarray(w).astype(bf),
        "u": np.ascontiguousarray(u).astype(bf),
        "bias": np.ascontiguousarray(bias),
        "bhh": np.ascontiguousarray(bhh),
    }


def kernel(tokens, emb, Wf, Uf, bif, bhf, Wb, Ub, bib, bhb):
    tokens = np.asarray(tokens)
    emb = np.asarray(emb, dtype=np.float32)
    Wf, Uf, Wb, Ub = (np.asarray(a, dtype=np.float32) for a in (Wf, Uf, Wb, Ub))
    bif, bhf, bib, bhb = (np.asarray(a, dtype=np.float32) for a in (bif, bhf, bib, bhb))

    tok_rev = tokens[:, ::-1].astype(np.int64)
    x = emb[tok_rev]                                           # [B, T, E] f32

    if "nc" not in _CACHE:
        _CACHE["nc"] = _build_program()
    nc = _CACHE["nc"]

    in_maps = []
    for c in range(8):
        fwd = c < 4
        bsl = slice((c % 4) * BC, (c % 4 + 1) * BC)
        xc = x[bsl] if fwd else x[bsl, ::-1]
        if fwd:
            in_maps.append(_prep_core_inputs(xc, Wf, Uf, bif, bhf))
        else:
            in_maps.append(_prep_core_inputs(xc, Wb, Ub, bib, bhb))

    trace = bool(int(os.environ.get("GRU_TRACE", "0")))
    res = run_bass_kernel_spmd(nc, in_maps, core_ids=list(range(8)), trace=trace)
    if trace and res.exec_time_ns is not None:
        print(f"HW exec time: {res.exec_time_ns} ns")
        _CACHE["exec_time_ns"] = res.exec_time_ns

    outputs = np.zeros((B, T, U), np.float32)
    hidden = np.zeros((B, 2 * U), np.float32)
    for c in range(8):
        o = res.results[c]["outs"]                             # [T, 128, KU, BC]
        o = o.transpose(3, 0, 2, 1).reshape(BC, T, U)          # [BC, T, U]
        bsl = slice((c % 4) * BC, (c % 4 + 1) * BC)
        if c < 4:
            outputs[bsl] += o
            hidden[bsl, :U] = o[:, T - 1, :]
        else:
            outputs[bsl] += o[:, ::-1, :]
            hidden[bsl, U:] = o[:, T - 1, :]
    return outputs, hidden
